# revision 3
# baseline (speedup 1.0000x reference)
"""Trainium2 Bass kernel for LinearPotential (RBF potential evaluation).

out[n] = sum_m c_m * exp(-||x_n - a_m||^2 * w_m),  w_m = 0.5 / p_m^2

Strategy (data-parallel over the 8 NeuronCores, points sharded, anchors
replicated — no collectives):

  arg[n,m] = 2w(a.x) - w*x_sq - w*a_sq + ln|c|      (fold |c| into the exp)
           = sum_k  P[k,n] * R[k,m]                 (K-row contraction)

  - TensorE: the contraction is evaluated as a matmul with points on the
    output-partition axis and anchors on the free axis. Full fp32 matmul is
    4x slow and fp32r is only ~2^-12 accurate, so each fp32 factor is split
    into 3 bf16 components and each scalar product is expanded into 6
    partial-product rows (errors ~2^-26 relative) => K = 4*6 + 3 = 27 bf16
    rows, which still streams at 1 column/cycle.
  - ScalarE: exp() + free-dim accumulation in a single ACTIVATE.  Anchors
    are permuted so positive coefficients come first: one ACTIVATE+accum per
    sign block, result = pos_accum - neg_accum (exp is positive; the sign
    cannot be folded into the exponent).
  - VectorE: the final [128, 128] subtract.

Self-contained: hardcodes shapes for N=131072 points, M=2048 anchors.
"""

import numpy as np
import ml_dtypes

import concourse.tile as tile
from concourse import bacc, mybir
from concourse.bass_utils import run_bass_kernel_spmd

N_CORES = 8
N_POINTS = 131072
N_ANCH = 2048
N_LOC = N_POINTS // N_CORES  # 16384 points per core
P = 128                      # partition dim / points per tile
N_TILES = N_LOC // P         # 128 tiles per core
K_ROWS = 27                  # 4 products x 6 split rows + 3 const rows
MM_N = 512                   # matmul free-dim tile (one PSUM bank, fp32)

_BF16 = ml_dtypes.bfloat16

_program_cache: dict = {}

# test-harness hooks (no effect on grading: default off)
TRACE = False
LAST_RESULTS = None


def _split3(v: np.ndarray):
    """Split fp64 array into 3 bf16 components h+m+l ~ v (rel err ~2^-27)."""
    h = v.astype(_BF16)
    r = v - h.astype(np.float64)
    m = r.astype(_BF16)
    r2 = r - m.astype(np.float64)
    l = r2.astype(_BF16)
    return h, m, l


def _product_rows(u64: np.ndarray, v64: np.ndarray):
    """Rows for an accurate scalar product u*v via 6 bf16 partial products.

    Returns (point_rows, anchor_rows): lists of 6 bf16 vectors each such that
    sum_i point_rows[i] (x) anchor_rows[i] ~= u (x) v with ~2^-26 rel error.
    """
    uh, um, ul = _split3(u64)
    vh, vm, vl = _split3(v64)
    return [uh, uh, um, um, uh, ul], [vh, vm, vh, vm, vl, vh]


def _build_program(m_pos: int):
    """Build + compile the per-core Bass program (same on all 8 cores)."""
    nc = bacc.Bacc("TRN2", target_bir_lowering=False, debug=False,
                   num_devices=N_CORES)
    pm_d = nc.dram_tensor("pm", [K_ROWS, N_LOC], mybir.dt.bfloat16,
                          kind="ExternalInput").ap()
    r_d = nc.dram_tensor("r", [K_ROWS, N_ANCH], mybir.dt.bfloat16,
                         kind="ExternalInput").ap()
    out_d = nc.dram_tensor("out", [N_LOC], mybir.dt.float32,
                           kind="ExternalOutput").ap()

    exp_f = mybir.ActivationFunctionType.Exp
    with tile.TileContext(nc) as tc:
        with (
            tc.tile_pool(name="const", bufs=1) as cpool,
            tc.tile_pool(name="scratch", bufs=2) as spool,
            tc.tile_pool(name="psum", bufs=2, space="PSUM") as ppool,
        ):
            pm = cpool.tile([K_ROWS, N_LOC], mybir.dt.bfloat16)
            rr = cpool.tile([K_ROWS, N_ANCH], mybir.dt.bfloat16)
            nc.sync.dma_start(pm[:], pm_d[:])
            nc.sync.dma_start(rr[:], r_d[:])

            pos = cpool.tile([P, N_TILES], mybir.dt.float32)
            neg = cpool.tile([P, N_TILES], mybir.dt.float32)
            res = cpool.tile([P, N_TILES], mybir.dt.float32)
            if m_pos == 0:
                nc.vector.memset(pos[:], 0.0)
            if m_pos == N_ANCH:
                nc.vector.memset(neg[:], 0.0)

            for i in range(N_TILES):
                ps = ppool.tile([P, N_ANCH], mybir.dt.float32)
                lhsT = pm[:, P * i : P * (i + 1)]
                for j in range(N_ANCH // MM_N):
                    nc.tensor.matmul(
                        ps[:, MM_N * j : MM_N * (j + 1)],
                        lhsT=lhsT,
                        rhs=rr[:, MM_N * j : MM_N * (j + 1)],
                        start=True,
                        stop=True,
                    )
                sc = spool.tile([P, N_ANCH], mybir.dt.bfloat16)
                if m_pos > 0:
                    nc.scalar.activation(
                        sc[:, 0:m_pos], ps[:, 0:m_pos], exp_f,
                        accum_out=pos[:, i : i + 1],
                    )
                if m_pos < N_ANCH:
                    nc.scalar.activation(
                        sc[:, m_pos:N_ANCH], ps[:, m_pos:N_ANCH], exp_f,
                        accum_out=neg[:, i : i + 1],
                    )
            nc.vector.tensor_sub(res[:], pos[:], neg[:])
            nc.sync.dma_start(out_d.rearrange("(p i) -> p i", i=N_TILES), res[:])
    nc.compile()
    return nc


def _prep_host(locations3d, anchor_locations3d, anchor_coeffs,
               anchor_parameters):
    """Build the 27-row point/anchor factor matrices (fp64 -> bf16 splits)."""
    x64 = locations3d.astype(np.float64)            # [N, 3]
    a64 = anchor_locations3d.astype(np.float64)     # [M, 3]
    c64 = anchor_coeffs.astype(np.float64)          # [M]
    p64 = anchor_parameters.astype(np.float64)      # [M]

    w = 0.5 / (p64 * p64)                           # [M]
    a_sq = (a64 * a64).sum(axis=1)                  # [M]
    x_sq = (x64 * x64).sum(axis=1)                  # [N]

    # permute anchors: positive coeffs first
    order = np.argsort(c64 <= 0, kind="stable")     # False(=pos) first
    m_pos = int((c64 > 0).sum())
    a64 = a64[order]
    c64 = c64[order]
    w = w[order]
    a_sq = a_sq[order]

    ln_c = np.log(np.maximum(np.abs(c64), 1e-300))
    ln_c = np.maximum(ln_c, -60.0)                  # exp(-60) ~ 9e-27 ~ 0

    # anchor-side factors F_t and point-side factors u_t:
    #   arg = sum_c x_c*(2 w a_c) + x_sq*(-w) + 1*(-w a_sq + ln|c|)
    point_factors = [x64[:, 0], x64[:, 1], x64[:, 2], x_sq]
    anchor_factors = [2.0 * w * a64[:, 0], 2.0 * w * a64[:, 1],
                      2.0 * w * a64[:, 2], -w]
    const_anchor = -w * a_sq + ln_c

    p_rows, r_rows = [], []
    for u, v in zip(point_factors, anchor_factors):
        pr, rr = _product_rows(u, v)
        p_rows.extend(pr)
        r_rows.extend(rr)
    ch, cm, cl = _split3(const_anchor)
    ones = np.ones(x_sq.shape[0], dtype=_BF16)
    p_rows.extend([ones, ones, ones])
    r_rows.extend([ch, cm, cl])

    P27 = np.stack(p_rows).astype(_BF16)            # [27, N]
    R27 = np.stack(r_rows).astype(_BF16)            # [27, M]
    return P27, R27, m_pos


def kernel(locations3d, anchor_locations3d, anchor_coeffs, anchor_parameters):
    assert locations3d.shape == (N_POINTS, 3)
    assert anchor_locations3d.shape == (N_ANCH, 3)

    P27, R27, m_pos = _prep_host(
        locations3d, anchor_locations3d, anchor_coeffs, anchor_parameters
    )

    nc = _program_cache.get(m_pos)
    if nc is None:
        nc = _build_program(m_pos)
        _program_cache[m_pos] = nc

    in_maps = []
    for c in range(N_CORES):
        shard = P27[:, c * N_LOC : (c + 1) * N_LOC]
        # reorder columns so tile i column p holds local point 128p + i:
        # the accum layout then DMAs out contiguously per partition.
        shard = np.ascontiguousarray(
            shard.reshape(K_ROWS, N_TILES, P).transpose(0, 2, 1)
            .reshape(K_ROWS, N_LOC)
        )
        in_maps.append({"pm": shard, "r": R27})

    res = run_bass_kernel_spmd(
        nc, in_maps, core_ids=list(range(N_CORES)), trace=TRACE
    )
    global LAST_RESULTS
    LAST_RESULTS = res
    out = np.concatenate([res.results[c]["out"] for c in range(N_CORES)])
    return out.astype(np.float32)


# revision 5
# speedup vs baseline: 1.7428x; 1.7428x over previous
"""Trainium2 Bass kernel for LinearPotential (RBF potential evaluation).

out[n] = sum_m c_m * exp(-||x_n - a_m||^2 * w_m),  w_m = 0.5 / p_m^2

Strategy (data-parallel over the 8 NeuronCores, points sharded, anchors
replicated — no collectives):

  arg[n,m] = 2w(a.x) - w*x_sq - w*a_sq + ln|c|      (fold |c| into the exp)
           = sum_k  P[k,n] * R[k,m]                 (K-row contraction)

  - TensorE: the contraction is evaluated as a matmul with points on the
    output-partition axis and anchors on the free axis. Full fp32 matmul is
    4x slow and fp32r is only ~2^-12 accurate, so each fp32 factor is split
    into 3 bf16 components and each scalar product is expanded into 6
    partial-product rows (errors ~2^-26 relative) => K = 4*6 + 3 = 27 bf16
    rows, which still streams at 1 column/cycle.
  - ScalarE: exp() + free-dim accumulation in a single ACTIVATE.  Anchors
    are permuted so positive coefficients come first: one ACTIVATE+accum per
    sign block, result = pos_accum - neg_accum (exp is positive; the sign
    cannot be folded into the exponent).
  - VectorE: the final [128, 128] subtract.

Self-contained: hardcodes shapes for N=131072 points, M=2048 anchors.
"""

import numpy as np
import ml_dtypes

import concourse.tile as tile
from concourse import bacc, mybir
from concourse.bass_utils import run_bass_kernel_spmd

N_CORES = 8
N_POINTS = 131072
N_ANCH = 2048
N_LOC = N_POINTS // N_CORES  # 16384 points per core
P = 128                      # partition dim / points per tile
N_TILES = N_LOC // P         # 128 tiles per core
K_ROWS = 27                  # 4 products x 6 split rows + 3 const rows
MM_N = 512                   # matmul free-dim tile (one PSUM bank, fp32)

_BF16 = ml_dtypes.bfloat16

_program_cache: dict = {}

# test-harness hooks (no effect on grading: default off)
TRACE = False
LAST_RESULTS = None


def _split3(v: np.ndarray):
    """Split fp64 array into 3 bf16 components h+m+l ~ v (rel err ~2^-27)."""
    h = v.astype(_BF16)
    r = v - h.astype(np.float64)
    m = r.astype(_BF16)
    r2 = r - m.astype(np.float64)
    l = r2.astype(_BF16)
    return h, m, l


def _product_rows(u64: np.ndarray, v64: np.ndarray):
    """Rows for an accurate scalar product u*v via 6 bf16 partial products.

    Returns (point_rows, anchor_rows): lists of 6 bf16 vectors each such that
    sum_i point_rows[i] (x) anchor_rows[i] ~= u (x) v with ~2^-26 rel error.
    """
    uh, um, ul = _split3(u64)
    vh, vm, vl = _split3(v64)
    return [uh, uh, um, um, uh, ul], [vh, vm, vh, vm, vl, vh]


def _build_program(m_pos: int):
    """Build + compile the per-core Bass program (same on all 8 cores)."""
    nc = bacc.Bacc("TRN2", target_bir_lowering=False, debug=False,
                   num_devices=N_CORES)
    pm_d = nc.dram_tensor("pm", [K_ROWS, N_LOC], mybir.dt.bfloat16,
                          kind="ExternalInput").ap()
    r_d = nc.dram_tensor("r", [K_ROWS, N_ANCH], mybir.dt.bfloat16,
                         kind="ExternalInput").ap()
    out_d = nc.dram_tensor("out", [N_LOC], mybir.dt.float32,
                           kind="ExternalOutput").ap()

    exp_f = mybir.ActivationFunctionType.Exp
    with tile.TileContext(nc) as tc:
        with (
            tc.tile_pool(name="const", bufs=1) as cpool,
            tc.tile_pool(name="scratch", bufs=3) as spool,
            tc.tile_pool(name="psum", bufs=2, space="PSUM") as ppool,
        ):
            pm = cpool.tile([K_ROWS, N_LOC], mybir.dt.bfloat16)
            rr = cpool.tile([K_ROWS, N_ANCH], mybir.dt.bfloat16)
            nc.sync.dma_start(pm[:], pm_d[:])
            nc.sync.dma_start(rr[:], r_d[:])

            sall = cpool.tile([P, N_TILES], mybir.dt.float32)
            negs = cpool.tile([P, N_TILES], mybir.dt.float32)
            res = cpool.tile([P, N_TILES], mybir.dt.float32)
            if m_pos == N_ANCH:
                nc.vector.memset(negs[:], 0.0)

            for i in range(N_TILES):
                ps = ppool.tile([P, N_ANCH], mybir.dt.float32)
                lhsT = pm[:, P * i : P * (i + 1)]
                for j in range(N_ANCH // MM_N):
                    nc.tensor.matmul(
                        ps[:, MM_N * j : MM_N * (j + 1)],
                        lhsT=lhsT,
                        rhs=rr[:, MM_N * j : MM_N * (j + 1)],
                        start=True,
                        stop=True,
                    )
                # One Exp ACTIVATE over the full anchor range; the hardware
                # accumulator gives S_all = sum_m |c| e^arg. The elementwise
                # output lands in fp16 scratch, from which VectorE re-sums
                # just the negative-coefficient block: out = S_all - 2*S_neg.
                sc = spool.tile([P, N_ANCH], mybir.dt.float16)
                nc.scalar.activation(
                    sc[:], ps[:], exp_f, accum_out=sall[:, i : i + 1]
                )
                if m_pos < N_ANCH:
                    nc.vector.reduce_sum(
                        negs[:, i : i + 1], sc[:, m_pos:N_ANCH],
                        axis=mybir.AxisListType.X,
                    )
            nc.vector.scalar_tensor_tensor(
                res[:], negs[:], -2.0, sall[:],
                mybir.AluOpType.mult, mybir.AluOpType.add,
            )
            nc.sync.dma_start(out_d.rearrange("(p i) -> p i", i=N_TILES), res[:])
    nc.compile()
    return nc


def _prep_host(locations3d, anchor_locations3d, anchor_coeffs,
               anchor_parameters):
    """Build the 27-row point/anchor factor matrices (fp64 -> bf16 splits)."""
    x64 = locations3d.astype(np.float64)            # [N, 3]
    a64 = anchor_locations3d.astype(np.float64)     # [M, 3]
    c64 = anchor_coeffs.astype(np.float64)          # [M]
    p64 = anchor_parameters.astype(np.float64)      # [M]

    w = 0.5 / (p64 * p64)                           # [M]
    a_sq = (a64 * a64).sum(axis=1)                  # [M]
    x_sq = (x64 * x64).sum(axis=1)                  # [N]

    # permute anchors: positive coeffs first
    order = np.argsort(c64 <= 0, kind="stable")     # False(=pos) first
    m_pos = int((c64 > 0).sum())
    a64 = a64[order]
    c64 = c64[order]
    w = w[order]
    a_sq = a_sq[order]

    ln_c = np.log(np.maximum(np.abs(c64), 1e-300))
    ln_c = np.maximum(ln_c, -60.0)                  # exp(-60) ~ 9e-27 ~ 0

    # anchor-side factors F_t and point-side factors u_t:
    #   arg = sum_c x_c*(2 w a_c) + x_sq*(-w) + 1*(-w a_sq + ln|c|)
    point_factors = [x64[:, 0], x64[:, 1], x64[:, 2], x_sq]
    anchor_factors = [2.0 * w * a64[:, 0], 2.0 * w * a64[:, 1],
                      2.0 * w * a64[:, 2], -w]
    const_anchor = -w * a_sq + ln_c

    p_rows, r_rows = [], []
    for u, v in zip(point_factors, anchor_factors):
        pr, rr = _product_rows(u, v)
        p_rows.extend(pr)
        r_rows.extend(rr)
    ch, cm, cl = _split3(const_anchor)
    ones = np.ones(x_sq.shape[0], dtype=_BF16)
    p_rows.extend([ones, ones, ones])
    r_rows.extend([ch, cm, cl])

    P27 = np.stack(p_rows).astype(_BF16)            # [27, N]
    R27 = np.stack(r_rows).astype(_BF16)            # [27, M]
    return P27, R27, m_pos


def kernel(locations3d, anchor_locations3d, anchor_coeffs, anchor_parameters):
    assert locations3d.shape == (N_POINTS, 3)
    assert anchor_locations3d.shape == (N_ANCH, 3)

    P27, R27, m_pos = _prep_host(
        locations3d, anchor_locations3d, anchor_coeffs, anchor_parameters
    )

    nc = _program_cache.get(m_pos)
    if nc is None:
        nc = _build_program(m_pos)
        _program_cache[m_pos] = nc

    in_maps = []
    for c in range(N_CORES):
        shard = P27[:, c * N_LOC : (c + 1) * N_LOC]
        # reorder columns so tile i column p holds local point 128p + i:
        # the accum layout then DMAs out contiguously per partition.
        shard = np.ascontiguousarray(
            shard.reshape(K_ROWS, N_TILES, P).transpose(0, 2, 1)
            .reshape(K_ROWS, N_LOC)
        )
        in_maps.append({"pm": shard, "r": R27})

    res = run_bass_kernel_spmd(
        nc, in_maps, core_ids=list(range(N_CORES)), trace=TRACE
    )
    global LAST_RESULTS
    LAST_RESULTS = res
    out = np.concatenate([res.results[c]["out"] for c in range(N_CORES)])
    return out.astype(np.float32)


# revision 6
# speedup vs baseline: 1.7771x; 1.0197x over previous
"""Trainium2 Bass kernel for LinearPotential (RBF potential evaluation).

out[n] = sum_m c_m * exp(-||x_n - a_m||^2 * w_m),  w_m = 0.5 / p_m^2

Strategy (data-parallel over the 8 NeuronCores, points sharded, anchors
replicated — no collectives):

  arg[n,m] = 2w(a.x) - w*x_sq - w*a_sq + ln|c|      (fold |c| into the exp)
           = sum_k  P[k,n] * R[k,m]                 (K-row contraction)

  - TensorE: the contraction is evaluated as a matmul with points on the
    output-partition axis and anchors on the free axis. Full fp32 matmul is
    4x slow and fp32r is only ~2^-12 accurate, so each fp32 factor is split
    into 3 bf16 components and each scalar product is expanded into 6
    partial-product rows (errors ~2^-26 relative) => K = 4*6 + 3 = 27 bf16
    rows, which still streams at 1 column/cycle.
  - ScalarE: exp() + free-dim accumulation in a single ACTIVATE.  Anchors
    are permuted so positive coefficients come first: one ACTIVATE+accum per
    sign block, result = pos_accum - neg_accum (exp is positive; the sign
    cannot be folded into the exponent).
  - VectorE: the final [128, 128] subtract.

Self-contained: hardcodes shapes for N=131072 points, M=2048 anchors.
"""

import numpy as np
import ml_dtypes

import concourse.tile as tile
from concourse import bacc, mybir
from concourse.bass_utils import run_bass_kernel_spmd

N_CORES = 8
N_POINTS = 131072
N_ANCH = 2048
N_LOC = N_POINTS // N_CORES  # 16384 points per core
P = 128                      # partition dim / points per tile
N_TILES = N_LOC // P         # 128 tiles per core
K_ROWS = 27                  # 4 products x 6 split rows + 3 const rows
MM_N = 512                   # matmul free-dim tile (one PSUM bank, fp32)

_BF16 = ml_dtypes.bfloat16

_program_cache: dict = {}

# test-harness hooks (no effect on grading: default off)
TRACE = False
LAST_RESULTS = None


def _split3(v: np.ndarray):
    """Split fp64 array into 3 bf16 components h+m+l ~ v (rel err ~2^-27)."""
    h = v.astype(_BF16)
    r = v - h.astype(np.float64)
    m = r.astype(_BF16)
    r2 = r - m.astype(np.float64)
    l = r2.astype(_BF16)
    return h, m, l


def _product_rows(u64: np.ndarray, v64: np.ndarray):
    """Rows for an accurate scalar product u*v via 6 bf16 partial products.

    Returns (point_rows, anchor_rows): lists of 6 bf16 vectors each such that
    sum_i point_rows[i] (x) anchor_rows[i] ~= u (x) v with ~2^-26 rel error.
    """
    uh, um, ul = _split3(u64)
    vh, vm, vl = _split3(v64)
    return [uh, uh, um, um, uh, ul], [vh, vm, vh, vm, vl, vh]


def _build_program(m_pos: int):
    """Build + compile the per-core Bass program (same on all 8 cores)."""
    nc = bacc.Bacc("TRN2", target_bir_lowering=False, debug=False,
                   num_devices=N_CORES)
    pm_d = nc.dram_tensor("pm", [K_ROWS, N_LOC], mybir.dt.bfloat16,
                          kind="ExternalInput").ap()
    r_d = nc.dram_tensor("r", [K_ROWS, N_ANCH], mybir.dt.bfloat16,
                         kind="ExternalInput").ap()
    out_d = nc.dram_tensor("out", [N_LOC], mybir.dt.float32,
                           kind="ExternalOutput").ap()

    exp_f = mybir.ActivationFunctionType.Exp
    with tile.TileContext(nc) as tc:
        with (
            tc.tile_pool(name="const", bufs=1) as cpool,
            tc.tile_pool(name="scratch", bufs=3) as spool,
            tc.tile_pool(name="psum", bufs=2, space="PSUM") as ppool,
        ):
            pm = cpool.tile([K_ROWS, N_LOC], mybir.dt.bfloat16)
            rr = cpool.tile([K_ROWS, N_ANCH], mybir.dt.bfloat16)
            nc.sync.dma_start(rr[:], r_d[:])
            # chunked point-matrix load so the first matmuls start early
            n_chunks = 16
            cw = N_LOC // n_chunks
            for c in range(n_chunks):
                nc.sync.dma_start(
                    pm[:, c * cw : (c + 1) * cw], pm_d[:, c * cw : (c + 1) * cw]
                )

            sall = cpool.tile([P, N_TILES], mybir.dt.float32)
            negs = cpool.tile([P, N_TILES], mybir.dt.float32)
            res = cpool.tile([P, N_TILES], mybir.dt.float32)
            if m_pos == N_ANCH:
                nc.vector.memset(negs[:], 0.0)

            for i in range(N_TILES):
                ps = ppool.tile([P, N_ANCH], mybir.dt.float32)
                lhsT = pm[:, P * i : P * (i + 1)]
                for j in range(N_ANCH // MM_N):
                    nc.tensor.matmul(
                        ps[:, MM_N * j : MM_N * (j + 1)],
                        lhsT=lhsT,
                        rhs=rr[:, MM_N * j : MM_N * (j + 1)],
                        start=True,
                        stop=True,
                    )
                # One Exp ACTIVATE over the full anchor range; the hardware
                # accumulator gives S_all = sum_m |c| e^arg. The elementwise
                # output lands in fp16 scratch, from which VectorE re-sums
                # just the negative-coefficient block: out = S_all - 2*S_neg.
                sc = spool.tile([P, N_ANCH], mybir.dt.float16)
                nc.scalar.activation(
                    sc[:], ps[:], exp_f, accum_out=sall[:, i : i + 1]
                )
                if m_pos < N_ANCH:
                    nc.vector.reduce_sum(
                        negs[:, i : i + 1], sc[:, m_pos:N_ANCH],
                        axis=mybir.AxisListType.X,
                    )
            nc.vector.scalar_tensor_tensor(
                res[:], negs[:], -2.0, sall[:],
                mybir.AluOpType.mult, mybir.AluOpType.add,
            )
            nc.sync.dma_start(out_d.rearrange("(p i) -> p i", i=N_TILES), res[:])
    nc.compile()
    return nc


def _prep_host(locations3d, anchor_locations3d, anchor_coeffs,
               anchor_parameters):
    """Build the 27-row point/anchor factor matrices (fp64 -> bf16 splits)."""
    x64 = locations3d.astype(np.float64)            # [N, 3]
    a64 = anchor_locations3d.astype(np.float64)     # [M, 3]
    c64 = anchor_coeffs.astype(np.float64)          # [M]
    p64 = anchor_parameters.astype(np.float64)      # [M]

    w = 0.5 / (p64 * p64)                           # [M]
    a_sq = (a64 * a64).sum(axis=1)                  # [M]
    x_sq = (x64 * x64).sum(axis=1)                  # [N]

    # permute anchors: positive coeffs first
    order = np.argsort(c64 <= 0, kind="stable")     # False(=pos) first
    m_pos = int((c64 > 0).sum())
    a64 = a64[order]
    c64 = c64[order]
    w = w[order]
    a_sq = a_sq[order]

    ln_c = np.log(np.maximum(np.abs(c64), 1e-300))
    ln_c = np.maximum(ln_c, -60.0)                  # exp(-60) ~ 9e-27 ~ 0

    # anchor-side factors F_t and point-side factors u_t:
    #   arg = sum_c x_c*(2 w a_c) + x_sq*(-w) + 1*(-w a_sq + ln|c|)
    point_factors = [x64[:, 0], x64[:, 1], x64[:, 2], x_sq]
    anchor_factors = [2.0 * w * a64[:, 0], 2.0 * w * a64[:, 1],
                      2.0 * w * a64[:, 2], -w]
    const_anchor = -w * a_sq + ln_c

    p_rows, r_rows = [], []
    for u, v in zip(point_factors, anchor_factors):
        pr, rr = _product_rows(u, v)
        p_rows.extend(pr)
        r_rows.extend(rr)
    ch, cm, cl = _split3(const_anchor)
    ones = np.ones(x_sq.shape[0], dtype=_BF16)
    p_rows.extend([ones, ones, ones])
    r_rows.extend([ch, cm, cl])

    P27 = np.stack(p_rows).astype(_BF16)            # [27, N]
    R27 = np.stack(r_rows).astype(_BF16)            # [27, M]
    return P27, R27, m_pos


def kernel(locations3d, anchor_locations3d, anchor_coeffs, anchor_parameters):
    assert locations3d.shape == (N_POINTS, 3)
    assert anchor_locations3d.shape == (N_ANCH, 3)

    P27, R27, m_pos = _prep_host(
        locations3d, anchor_locations3d, anchor_coeffs, anchor_parameters
    )

    nc = _program_cache.get(m_pos)
    if nc is None:
        nc = _build_program(m_pos)
        _program_cache[m_pos] = nc

    in_maps = []
    for c in range(N_CORES):
        shard = P27[:, c * N_LOC : (c + 1) * N_LOC]
        # reorder columns so tile i column p holds local point 128p + i:
        # the accum layout then DMAs out contiguously per partition.
        shard = np.ascontiguousarray(
            shard.reshape(K_ROWS, N_TILES, P).transpose(0, 2, 1)
            .reshape(K_ROWS, N_LOC)
        )
        in_maps.append({"pm": shard, "r": R27})

    res = run_bass_kernel_spmd(
        nc, in_maps, core_ids=list(range(N_CORES)), trace=TRACE
    )
    global LAST_RESULTS
    LAST_RESULTS = res
    out = np.concatenate([res.results[c]["out"] for c in range(N_CORES)])
    return out.astype(np.float32)
